# revision 1
# baseline (speedup 1.0000x reference)
"""MinEntropyConsensusLoss Trainium2 kernel.

ce = 0.5 * mean_b( min_c( -log_softmax(x)[b,c] - log_softmax(y)[b,c] ) )
   = 0.5 * mean_b( lse(x_b) + lse(y_b) - max_c(x[b,c] + y[b,c]) )

Data-parallel over 8 NeuronCores; each streams 16384 rows of x and y and
emits per-row stats: sum(exp(x)), sum(exp(y)) and max(x+y). Host applies
log and the global mean in float64 (permutation-invariant, so row->slot
mapping never needs to be undone).

Engine split (measured per 128-row group, DMA budget ~710ns):
  ACT    exp(y) in 2048-col batches; exp(x) half solo+fused-accum (row
         sums), half batched                      ~640ns
  GPSIMD x+y elementwise add, full 4096-col chunk ~700ns
  DVE    3D-batched reduce_max(x+y) + reduce_sum(exp) into stat tiles
                                                  ~670ns
"""

import sys

sys.path.insert(0, "/opt/trn_rl_repo")

import numpy as np

import concourse.bacc as bacc
import concourse.mybir as mybir
import concourse.tile as tile
from concourse.bass_utils import run_bass_kernel_spmd

B, C = 131072, 256
N_CORES = 8
R = B // N_CORES          # rows per core = 16384
T = 8                     # rows per partition per chunk
NG = R // 128             # 128 row-groups per core
NACC = 4                  # groups per chunk whose sum(exp(x)) uses ACT accum

_cache = {}


def _build_nc(repeat=1, nacc=NACC, bufs=4, nsplit=2, tt=T, dvadd=0, inbufs=None):
    f32 = mybir.dt.float32
    A = mybir.AluOpType
    Exp = mybir.ActivationFunctionType.Exp
    X = mybir.AxisListType.X
    T = tt
    NCH = R // (128 * T)
    nc = bacc.Bacc("TRN2", target_bir_lowering=False, debug=False)
    x_d = nc.dram_tensor("x", [R, C], f32, kind="ExternalInput")
    y_d = nc.dram_tensor("y", [R, C], f32, kind="ExternalInput")
    sxa_d = nc.dram_tensor("sxa", [128, NG], f32, kind="ExternalOutput")
    sxd_d = nc.dram_tensor("sxd", [128, NG], f32, kind="ExternalOutput")
    sy_d = nc.dram_tensor("sy", [128, NG], f32, kind="ExternalOutput")
    mxy_d = nc.dram_tensor("mxy", [128, NG], f32, kind="ExternalOutput")

    # chunk c, partition p holds rows c*CHUNK_ROWS + p*T + t  (t contiguous)
    x_v = x_d.ap().rearrange("(c p t) f -> c p (t f)", c=NCH, p=128, t=T)
    y_v = y_d.ap().rearrange("(c p t) f -> c p (t f)", c=NCH, p=128, t=T)

    with tile.TileContext(nc) as tc:
        with (
            tc.tile_pool(name="xin", bufs=inbufs or bufs) as xin_pool,
            tc.tile_pool(name="yin", bufs=inbufs or bufs) as yin_pool,
            tc.tile_pool(name="expy", bufs=bufs) as expy_pool,
            tc.tile_pool(name="expx", bufs=bufs) as expx_pool,
            tc.tile_pool(name="xys", bufs=bufs) as xy_pool,
            tc.tile_pool(name="dead", bufs=2, space="PSUM") as dead_pool,
            tc.tile_pool(name="stats", bufs=1) as stats_pool,
        ):
            sxa_t = stats_pool.tile([128, NG], f32, tag="sxa")
            sxd_t = stats_pool.tile([128, NG], f32, tag="sxd")
            nc.vector.memset(sxa_t[:], 1.0)
            nc.vector.memset(sxd_t[:], 1.0)
            sy_t = stats_pool.tile([128, NG], f32, tag="sy")
            mxy_t = stats_pool.tile([128, NG], f32, tag="mxy")

            def one_pass():
                for c in range(NCH):
                    g0 = c * T
                    x_t = xin_pool.tile([128, T * C], f32, tag="x")
                    nc.sync.dma_start(x_t[:], x_v[c])
                    y_t = yin_pool.tile([128, T * C], f32, tag="y")
                    nc.sync.dma_start(y_t[:], y_v[c])

                    # --- ScalarE: exponentials ---
                    # x, groups 0..nacc-1: solo instructions with fused
                    # row-sum accumulate (main out is a dead store in PSUM)
                    for t in range(nacc):
                        dead = dead_pool.tile([128, C], f32, tag="dead")
                        nc.scalar.activation(
                            dead[:], x_t[:, t * C : (t + 1) * C], Exp,
                            accum_out=sxa_t[:, g0 + t : g0 + t + 1],
                        )
                    # x, groups nacc..T-1: one batched exp; sums via DVE
                    nx = T - nacc
                    ex = expx_pool.tile([128, nx * C], f32, tag="ex")
                    nc.scalar.activation(ex[:], x_t[:, nacc * C :], Exp)
                    nc.vector.reduce_sum(
                        sxd_t[:, g0 + nacc : g0 + T],
                        ex[:].rearrange("p (t f) -> p t f", t=nx),
                        axis=X,
                    )
                    # y: batched exp; sums via DVE. GPSIMD does x+y; DVE
                    # max-reduces it. nsplit controls sub-chunk granularity.
                    ey = expy_pool.tile([128, T * C], f32, tag="ey")
                    xy = xy_pool.tile([128, T * C], f32, tag="xy")
                    ts = T // nsplit
                    for s_ in range(nsplit):
                        a, b = s_ * ts * C, (s_ + 1) * ts * C
                        ga, gb = g0 + s_ * ts, g0 + (s_ + 1) * ts
                        nc.scalar.activation(ey[:, a:b], y_t[:, a:b], Exp)
                        nc.vector.reduce_sum(
                            sy_t[:, ga:gb],
                            ey[:, a:b].rearrange("p (t f) -> p t f", t=ts),
                            axis=X,
                        )
                        # x+y: optionally give the first `dvadd` groups of
                        # the chunk to DVE, the rest to GPSIMD
                        d0 = max(a, dvadd * C) if s_ == 0 else a
                        if s_ == 0 and dvadd > 0:
                            nc.vector.tensor_tensor(
                                out=xy[:, a : dvadd * C], in0=x_t[:, a : dvadd * C],
                                in1=y_t[:, a : dvadd * C], op=A.add,
                            )
                        if d0 < b:
                            nc.gpsimd.tensor_tensor(
                                out=xy[:, d0:b], in0=x_t[:, d0:b], in1=y_t[:, d0:b],
                                op=A.add,
                            )
                        nc.vector.reduce_max(
                            mxy_t[:, ga:gb],
                            xy[:, a:b].rearrange("p (t f) -> p t f", t=ts),
                            axis=X,
                        )

            if repeat > 1:
                with tc.For_i(0, repeat, 1):
                    one_pass()
            else:
                one_pass()

            nc.sync.dma_start(sxa_d.ap(), sxa_t[:])
            nc.sync.dma_start(sxd_d.ap(), sxd_t[:])
            nc.sync.dma_start(sy_d.ap(), sy_t[:])
            nc.sync.dma_start(mxy_d.ap(), mxy_t[:])

    nc.compile()
    return nc


def get_nc():
    if "nc" not in _cache:
        _cache["nc"] = _build_nc()
    return _cache["nc"]


def run_cores(x, y, trace=False, **kw):
    nc = get_nc()
    x = np.ascontiguousarray(np.asarray(x, dtype=np.float32))
    y = np.ascontiguousarray(np.asarray(y, dtype=np.float32))
    in_maps = [
        {"x": x[k * R : (k + 1) * R], "y": y[k * R : (k + 1) * R]}
        for k in range(N_CORES)
    ]
    return run_bass_kernel_spmd(nc, in_maps, list(range(N_CORES)), trace=trace, **kw)


def kernel(x, y):
    res = run_cores(x, y)
    total = 0.0
    for r in res.results:
        sx = r["sxa"].astype(np.float64) * r["sxd"].astype(np.float64)
        sy = r["sy"].astype(np.float64)
        mxy = r["mxy"].astype(np.float64)
        total += float(np.sum(np.log(sx) + np.log(sy) - mxy))
    return np.float32(0.5 * total / B)



# revision 3
# speedup vs baseline: 10.8869x; 10.8869x over previous
"""MinEntropyConsensusLoss Trainium2 kernel.

loss = 0.5 * mean_b( min_c( -log_softmax(x)[b,c] - log_softmax(y)[b,c] ) )
     = 0.5 * mean_b( lse(x_b) + lse(y_b) - max_c(x[b,c] + y[b,c]) )

Data-parallel over 8 NeuronCores: each core streams 16384 rows of x and
y as 16 chunks of [128 partitions x 8 rows x 256 cols] and reduces to a
single [128, 1] per-partition partial on device; the host sums 8x128
values and scales.

Engine split per chunk (8 row-groups of 128 rows; ~700ns/group DMA
budget at the ~358 GB/s HBM-per-core limit):
  ACT    4x solo exp(x)+fused row-sum accum (dead store in PSUM) +
         1 batched exp(x) over 4 groups + 1 batched exp(y) over all 8
  DVE    3D reduce_sum of batched exp tiles; 2x 3D reduce_max of x+y
  GPSIMD 2x tensor_tensor x+y add (4 groups each)
  tail   sx*sxd, Ln, add, sub mxy, reduce_sum -> [128,1], one tiny DMA

Measured ~90-94us/pass/core vs 120us baseline; DMA-only floor ~82-85us.

Hardware pitfalls found on TRN2 (hold for future edits):
  - vector.tensor_tensor_reduce with op1=max or min WEDGES the core
    (NRT_EXEC_UNIT_UNRECOVERABLE); only the qr.py-style op0=mult/op1=add
    shape is silicon-safe.  Use tensor_tensor add + reduce_max instead.
  - gpsimd.scalar_tensor_tensor with accum_out fails neuronxcc codegen.
  - fp32 identity-matmul on TensorE runs ~1/4 rate: not a viable x+y.
  - ACT instructions cost ~240ns fixed overhead each: batch aggressively
    and cap solo accum_out instructions (here 4 of 16 per chunk).
"""

import sys

sys.path.insert(0, "/opt/trn_rl_repo")

import numpy as np

import concourse.bacc as bacc
import concourse.mybir as mybir
import concourse.tile as tile
from concourse.bass_utils import run_bass_kernel_spmd

B, C = 131072, 256
N_CORES = 8
R = B // N_CORES          # rows per core = 16384
T = 8                     # rows per partition per chunk (8 groups/chunk)
NCH = R // (128 * T)      # 16 chunks per core
NG = R // 128             # 128 row-groups per core
NAX = 4                   # groups/chunk with solo exp(x)+accum on ACT
NB = 8                    # groups per batched-exp scratch tile
NBM = 4                   # groups per GPSIMD add / DVE max sub-batch
BUFS = 9                  # input tile ring depth
SBUFS = 7                 # scratch ring depth

_cache = {}


def _build_nc(repeat=1):
    f32 = mybir.dt.float32
    A = mybir.AluOpType
    Exp = mybir.ActivationFunctionType.Exp
    Ln = mybir.ActivationFunctionType.Ln
    X = mybir.AxisListType.X
    nc = bacc.Bacc("TRN2", target_bir_lowering=False, debug=False)
    x_d = nc.dram_tensor("x", [R, C], f32, kind="ExternalInput")
    y_d = nc.dram_tensor("y", [R, C], f32, kind="ExternalInput")
    out_d = nc.dram_tensor("out", [128, 1], f32, kind="ExternalOutput")

    # chunk c, partition p holds rows c*1024 + p*T + t (t contiguous)
    x_v = x_d.ap().rearrange("(c p t) f -> c p (t f)", c=NCH, p=128, t=T)
    y_v = y_d.ap().rearrange("(c p t) f -> c p (t f)", c=NCH, p=128, t=T)

    with tile.TileContext(nc) as tc:
        with (
            tc.tile_pool(name="xin", bufs=BUFS) as xin_pool,
            tc.tile_pool(name="yin", bufs=BUFS) as yin_pool,
            tc.tile_pool(name="scr", bufs=SBUFS) as scr_pool,
            tc.tile_pool(name="dead", bufs=2, space="PSUM") as dead_pool,
            tc.tile_pool(name="stats", bufs=1) as stats_pool,
        ):
            sxa_t = stats_pool.tile([128, NG], f32, tag="sxa")
            sxd_t = stats_pool.tile([128, NG], f32, tag="sxd")
            syd_t = stats_pool.tile([128, NG], f32, tag="syd")
            mxy_t = stats_pool.tile([128, NG], f32, tag="mxy")
            # sx = sxa * sxd in the tail: unwritten halves must be 1.0
            # (solo-accum covers groups 0..NAX-1, batched covers the rest)
            nc.vector.memset(sxa_t[:], 1.0)
            nc.vector.memset(sxd_t[:], 1.0)

            def one_pass():
                for c in range(NCH):
                    g0 = c * T
                    x_t = xin_pool.tile([128, T * C], f32, tag="x")
                    nc.sync.dma_start(x_t[:], x_v[c])
                    y_t = yin_pool.tile([128, T * C], f32, tag="y")
                    nc.sync.dma_start(y_t[:], y_v[c])

                    # --- ACT: exponentials (+ row sums for solo groups) ---
                    for t in range(NAX):
                        dx = dead_pool.tile([128, C], f32, tag="dx")
                        nc.scalar.activation(
                            dx[:], x_t[:, t * C : (t + 1) * C], Exp,
                            accum_out=sxa_t[:, g0 + t : g0 + t + 1],
                        )
                    for s in range(NAX, T, NB):
                        e = min(s + NB, T)
                        n = e - s
                        ex = scr_pool.tile([128, NB * C], f32, tag="s")
                        nc.scalar.activation(
                            ex[:, : n * C], x_t[:, s * C : e * C], Exp)
                        nc.vector.reduce_sum(
                            sxd_t[:, g0 + s : g0 + e],
                            ex[:, : n * C].rearrange("p (t f) -> p t f", t=n),
                            axis=X,
                        )
                    for s in range(0, T, NB):
                        e = min(s + NB, T)
                        n = e - s
                        ey = scr_pool.tile([128, NB * C], f32, tag="s")
                        nc.scalar.activation(
                            ey[:, : n * C], y_t[:, s * C : e * C], Exp)
                        nc.vector.reduce_sum(
                            syd_t[:, g0 + s : g0 + e],
                            ey[:, : n * C].rearrange("p (t f) -> p t f", t=n),
                            axis=X,
                        )

                    # --- max(x+y): GPSIMD add, DVE 3D reduce_max ---
                    for s in range(0, T, NBM):
                        e = min(s + NBM, T)
                        n = e - s
                        xy = scr_pool.tile([128, NB * C], f32, tag="s")
                        nc.gpsimd.tensor_tensor(
                            out=xy[:, : n * C], in0=x_t[:, s * C : e * C],
                            in1=y_t[:, s * C : e * C], op=A.add,
                        )
                        nc.vector.reduce_max(
                            mxy_t[:, g0 + s : g0 + e],
                            xy[:, : n * C].rearrange("p (t f) -> p t f", t=n),
                            axis=X,
                        )

            if repeat > 1:
                with tc.For_i(0, repeat, 1):
                    one_pass()
            else:
                one_pass()

            # --- device tail: [128, NG] stats -> [128, 1] partial sum ---
            sx_t = stats_pool.tile([128, NG], f32, tag="sx")
            lx_t = stats_pool.tile([128, NG], f32, tag="lx")
            ly_t = stats_pool.tile([128, NG], f32, tag="ly")
            ll_t = stats_pool.tile([128, NG], f32, tag="ll")
            lm_t = stats_pool.tile([128, NG], f32, tag="lm")
            o_t = stats_pool.tile([128, 1], f32, tag="o")
            nc.vector.tensor_tensor(out=sx_t[:], in0=sxa_t[:], in1=sxd_t[:], op=A.mult)
            nc.scalar.activation(lx_t[:], sx_t[:], Ln)
            nc.scalar.activation(ly_t[:], syd_t[:], Ln)
            nc.vector.tensor_tensor(out=ll_t[:], in0=lx_t[:], in1=ly_t[:], op=A.add)
            nc.vector.tensor_tensor(out=lm_t[:], in0=ll_t[:], in1=mxy_t[:], op=A.subtract)
            nc.vector.reduce_sum(o_t[:], lm_t[:], axis=X)
            nc.sync.dma_start(out_d.ap(), o_t[:])

    nc.compile()
    return nc


def get_nc():
    if "nc" not in _cache:
        _cache["nc"] = _build_nc()
    return _cache["nc"]


def run_cores(x, y, **kw):
    nc = get_nc()
    x = np.ascontiguousarray(np.asarray(x, dtype=np.float32))
    y = np.ascontiguousarray(np.asarray(y, dtype=np.float32))
    in_maps = [
        {"x": x[k * R : (k + 1) * R], "y": y[k * R : (k + 1) * R]}
        for k in range(N_CORES)
    ]
    return run_bass_kernel_spmd(nc, in_maps, list(range(N_CORES)), **kw)


def kernel(x, y):
    res = run_cores(x, y)
    total = 0.0
    for r in res.results:
        total += float(np.sum(r["out"].astype(np.float64)))
    return np.float32(0.5 * total / B)


# revision 7
# speedup vs baseline: 12.5770x; 1.1552x over previous
"""MinEntropyConsensusLoss Trainium2 kernel.

loss = 0.5 * mean_b( min_c( -log_softmax(x)[b,c] - log_softmax(y)[b,c] ) )
     = 0.5 * mean_b( lse(x_b) + lse(y_b) - max_c(x[b,c] + y[b,c]) )

Data-parallel over 8 NeuronCores: each core streams 16384 rows of x and
y as 16 chunks of [128 partitions x 8 rows x 256 cols] and reduces to a
single [128, 1] per-partition partial on device; the host sums 8x128
values and scales.

Engine split per chunk (8 row-groups of 128 rows; ~700ns/group DMA
budget at the ~358 GB/s HBM-per-core limit):
  ACT    4x solo exp(x)+fused row-sum accum (dead store in PSUM) +
         1 batched exp(x) over 4 groups + 1 batched exp(y) over all 8
  DVE    3D reduce_sum of batched exp tiles; 2x 3D reduce_max of x+y
  GPSIMD 2x tensor_tensor x+y add (4 groups each)
  tail   sx*sxd, Ln, add, sub mxy, reduce_sum -> [128,1], one tiny DMA

Measured ~90-94us/pass/core vs 120us baseline; DMA-only floor ~82-85us.

Hardware pitfalls found on TRN2 (hold for future edits):
  - vector.tensor_tensor_reduce with op1=max or min WEDGES the core
    (NRT_EXEC_UNIT_UNRECOVERABLE); only the qr.py-style op0=mult/op1=add
    shape is silicon-safe.  Use tensor_tensor add + reduce_max instead.
  - gpsimd.scalar_tensor_tensor with accum_out fails neuronxcc codegen.
  - fp32 identity-matmul on TensorE runs ~1/4 rate: not a viable x+y.
  - ACT instructions cost ~240ns fixed overhead each: batch aggressively
    and cap solo accum_out instructions (here 4 of 16 per chunk).
"""

import sys

sys.path.insert(0, "/opt/trn_rl_repo")

import numpy as np

import concourse.bacc as bacc
import concourse.mybir as mybir
import concourse.tile as tile
from concourse.bass_utils import run_bass_kernel_spmd

B, C = 131072, 256
N_CORES = 8
R = B // N_CORES          # rows per core = 16384
T = 8                     # rows per partition per chunk (8 groups/chunk)
NCH = R // (128 * T)      # 16 chunks per core
NG = R // 128             # 128 row-groups per core
NAX = 4                   # groups/chunk with solo exp(x)+accum on ACT
NAY = 1                   # groups/chunk with solo exp(y)+accum on ACT
NB = 8                    # groups per batched-exp scratch tile
NBM = 4                   # groups per GPSIMD add / DVE max sub-batch
BUFS = 10                 # input tile ring depth
SBUFS = 5                 # scratch ring depth

_cache = {}


def _build_nc(repeat=1):
    f32 = mybir.dt.float32
    A = mybir.AluOpType
    Exp = mybir.ActivationFunctionType.Exp
    Ln = mybir.ActivationFunctionType.Ln
    X = mybir.AxisListType.X
    nc = bacc.Bacc("TRN2", target_bir_lowering=False, debug=False)
    x_d = nc.dram_tensor("x", [R, C], f32, kind="ExternalInput")
    y_d = nc.dram_tensor("y", [R, C], f32, kind="ExternalInput")
    out_d = nc.dram_tensor("out", [128, 1], f32, kind="ExternalOutput")

    # chunk c, partition p holds rows c*1024 + p*T + t (t contiguous)
    x_v = x_d.ap().rearrange("(c p t) f -> c p (t f)", c=NCH, p=128, t=T)
    y_v = y_d.ap().rearrange("(c p t) f -> c p (t f)", c=NCH, p=128, t=T)

    with tile.TileContext(nc) as tc:
        with (
            tc.tile_pool(name="xin", bufs=BUFS) as xin_pool,
            tc.tile_pool(name="yin", bufs=BUFS) as yin_pool,
            tc.tile_pool(name="scr", bufs=SBUFS) as scr_pool,
            tc.tile_pool(name="dead", bufs=2, space="PSUM") as dead_pool,
            tc.tile_pool(name="stats", bufs=1) as stats_pool,
        ):
            sxa_t = stats_pool.tile([128, NG], f32, tag="sxa")
            sxd_t = stats_pool.tile([128, NG], f32, tag="sxd")
            sya_t = stats_pool.tile([128, NG], f32, tag="sya")
            syd_t = stats_pool.tile([128, NG], f32, tag="syd")
            mxy_t = stats_pool.tile([128, NG], f32, tag="mxy")
            # sx = sxa * sxd (and sy = sya * syd) in the tail: unwritten
            # halves must be 1.0 (solo-accum covers the first NAX/NAY
            # groups, batched sums cover the rest)
            nc.vector.memset(sxa_t[:], 1.0)
            nc.vector.memset(sxd_t[:], 1.0)
            nc.vector.memset(sya_t[:], 1.0)
            nc.vector.memset(syd_t[:], 1.0)

            def one_pass():
                for c in range(NCH):
                    g0 = c * T
                    # half-chunk interleaved DMAs: first halves of x and y
                    # land first, so the first GPSIMD add / ACT solo exps
                    # (groups 0-3 = first half) start half a chunk-DMA
                    # earlier via sub-tile deps. ~7-9us/pass vs whole-chunk.
                    h = T * C // 2
                    x_t = xin_pool.tile([128, T * C], f32, tag="x")
                    y_t = yin_pool.tile([128, T * C], f32, tag="y")
                    nc.sync.dma_start(x_t[:, :h], x_v[c][:, :h])
                    nc.sync.dma_start(y_t[:, :h], y_v[c][:, :h])
                    nc.sync.dma_start(x_t[:, h:], x_v[c][:, h:])
                    nc.sync.dma_start(y_t[:, h:], y_v[c][:, h:])

                    # --- max(x+y) first: GPSIMD add, DVE 3D reduce_max.
                    # Emitting these before the exps keeps DVE's maxes
                    # (dep: GPSIMD <- DMA only) ahead of the sums that
                    # wait on ACT — measured ~18µs/pass faster. ---
                    for s in range(0, T, NBM):
                        e = min(s + NBM, T)
                        n = e - s
                        xy = scr_pool.tile([128, NB * C], f32, tag="s")
                        nc.gpsimd.tensor_tensor(
                            out=xy[:, : n * C], in0=x_t[:, s * C : e * C],
                            in1=y_t[:, s * C : e * C], op=A.add,
                        )
                        nc.vector.reduce_max(
                            mxy_t[:, g0 + s : g0 + e],
                            xy[:, : n * C].rearrange("p (t f) -> p t f", t=n),
                            axis=X,
                        )

                    # --- ACT: exponentials (+ row sums for solo groups) ---
                    for t in range(NAX):
                        dx = dead_pool.tile([128, C], f32, tag="dx")
                        nc.scalar.activation(
                            dx[:], x_t[:, t * C : (t + 1) * C], Exp,
                            accum_out=sxa_t[:, g0 + t : g0 + t + 1],
                        )
                    for s in range(NAX, T, NB):
                        e = min(s + NB, T)
                        n = e - s
                        ex = scr_pool.tile([128, NB * C], f32, tag="s")
                        nc.scalar.activation(
                            ex[:, : n * C], x_t[:, s * C : e * C], Exp)
                        nc.vector.reduce_sum(
                            sxd_t[:, g0 + s : g0 + e],
                            ex[:, : n * C].rearrange("p (t f) -> p t f", t=n),
                            axis=X,
                        )
                    for t in range(NAY):
                        dy = dead_pool.tile([128, C], f32, tag="dy")
                        nc.scalar.activation(
                            dy[:], y_t[:, t * C : (t + 1) * C], Exp,
                            accum_out=sya_t[:, g0 + t : g0 + t + 1],
                        )
                    for s in range(NAY, T, NB):
                        e = min(s + NB, T)
                        n = e - s
                        ey = scr_pool.tile([128, NB * C], f32, tag="s")
                        nc.scalar.activation(
                            ey[:, : n * C], y_t[:, s * C : e * C], Exp)
                        nc.vector.reduce_sum(
                            syd_t[:, g0 + s : g0 + e],
                            ey[:, : n * C].rearrange("p (t f) -> p t f", t=n),
                            axis=X,
                        )

            if repeat > 1:
                with tc.For_i(0, repeat, 1):
                    one_pass()
            else:
                one_pass()

            # --- device tail: [128, NG] stats -> [128, 1] partial sum ---
            sx_t = stats_pool.tile([128, NG], f32, tag="sx")
            sy_t = stats_pool.tile([128, NG], f32, tag="sy")
            lx_t = stats_pool.tile([128, NG], f32, tag="lx")
            ly_t = stats_pool.tile([128, NG], f32, tag="ly")
            ll_t = stats_pool.tile([128, NG], f32, tag="ll")
            lm_t = stats_pool.tile([128, NG], f32, tag="lm")
            o_t = stats_pool.tile([128, 1], f32, tag="o")
            nc.vector.tensor_tensor(out=sx_t[:], in0=sxa_t[:], in1=sxd_t[:], op=A.mult)
            nc.vector.tensor_tensor(out=sy_t[:], in0=sya_t[:], in1=syd_t[:], op=A.mult)
            nc.scalar.activation(lx_t[:], sx_t[:], Ln)
            nc.scalar.activation(ly_t[:], sy_t[:], Ln)
            nc.vector.tensor_tensor(out=ll_t[:], in0=lx_t[:], in1=ly_t[:], op=A.add)
            nc.vector.tensor_tensor(out=lm_t[:], in0=ll_t[:], in1=mxy_t[:], op=A.subtract)
            nc.vector.reduce_sum(o_t[:], lm_t[:], axis=X)
            nc.sync.dma_start(out_d.ap(), o_t[:])

    nc.compile()
    return nc


def get_nc():
    if "nc" not in _cache:
        _cache["nc"] = _build_nc()
    return _cache["nc"]


def run_cores(x, y, **kw):
    nc = get_nc()
    x = np.ascontiguousarray(np.asarray(x, dtype=np.float32))
    y = np.ascontiguousarray(np.asarray(y, dtype=np.float32))
    in_maps = [
        {"x": x[k * R : (k + 1) * R], "y": y[k * R : (k + 1) * R]}
        for k in range(N_CORES)
    ]
    return run_bass_kernel_spmd(nc, in_maps, list(range(N_CORES)), **kw)


def kernel(x, y):
    res = run_cores(x, y)
    total = 0.0
    for r in res.results:
        total += float(np.sum(r["out"].astype(np.float64)))
    return np.float32(0.5 * total / B)
